# revision 5
# baseline (speedup 1.0000x reference)
"""Multi-head attention (B=8, H=8, S=1024, d=128) on 8 TRN2 NeuronCores.

Strategy (v2)
-------------
- Data-parallel over batch: core i handles batch i (8 cores, B=8).
- Host-side prep (layout only): per batch, compact keys/values to the
  seq_mask-selected rows (zero-padded to kt_tiles 128-wide k-tiles),
  pre-transpose Q and compacted K so the contraction dim (d) lands on
  SBUF partitions, cast matmul operands to fp16. An indicator matrix
  ind[k, 32] (1 for real keys) rides along for the softmax denominator.
- All inputs are bulk-preloaded into SBUF with a handful of large DMAs
  (head-0 slices first so compute starts early); no per-head DMA stalls.
- Device math per (head h, k-tile kt):
    logitsT[k, q]  = K^T[:, kt].T @ Q^T      (PE, fp16, bf16 PSUM out)
    W^T[k, q]      = exp(logitsT * d^-0.5)   (ACT exp LUT for ~3/5 tiles,
                     DVE fast-exp bit-trick (tensor_scalar -> int16 bits
                     of fp16) for ~2/5 tiles: splits the exp load across
                     two engines; |rel err| <= ~3% on the trick tiles,
                     which averages out across ~500 attended keys)
    outT[d, q]    += V[kt].T   @ W^T         (PE, fp32 PSUM accum)
    den[q]        += ind[kt].T @ W^T         (PE, 2 heads packed per
                                              PSUM bank via partitions)
  Outputs leave as fp16 (numerator and denominator); the division
  happens on the host. The learned scalar bias b cancels in softmax.
- Host-side unshard: out[b] = (numT / den).T (uniform-average fallback
  for a fully-masked batch).
"""
from contextlib import ExitStack

import numpy as np

import concourse.bacc as bacc
import concourse.mybir as mybir
import concourse.tile as tile
from concourse.bass_utils import run_bass_kernel_spmd

F32 = mybir.dt.float32
F16 = mybir.dt.float16
BF16 = mybir.dt.bfloat16
I16 = mybir.dt.int16

B, S, D, H = 8, 1024, 1024, 8
DH = D // H              # 128, head dim = one partition tile
SCALE = float(DH) ** -0.5

# fast-exp bit trick constants (fp16): exp(x) ~= fp16_from_bits(
#   round(x * 1024*log2(e) + (15*1024 - 44)))  -- 44 centers the
# log2(1+m)-m mantissa interpolation error (+-3%, ~zero mean).
EXP_C1 = 1024.0 * 1.4426950408889634 * SCALE
EXP_C2 = 15.0 * 1024.0 - 44.0

_NC_CACHE: dict[tuple, object] = {}

# build options (overridable for profiling experiments)
OPTS: dict = {}


def _build(kt_tiles: int, opts: dict | None = None):
    """Build + compile the per-core kernel for `kt_tiles` 128-wide key tiles."""
    opts = opts or {}
    KP = kt_tiles * 128
    pl_bufs = opts.get("pl_bufs", 3)
    po_bufs = opts.get("po_bufs", 2)
    # which k-tiles the DVE (bit-trick) exp handles
    dve_kts = opts.get("dve_kts", (1, 3))

    nc = bacc.Bacc("TRN2", target_bir_lowering=False, debug=False)

    q_t = nc.dram_tensor("q_t", [D, S], F16, kind="ExternalInput")
    k_t = nc.dram_tensor("k_t", [D, KP], F16, kind="ExternalInput")
    v_c = nc.dram_tensor("v_c", [KP, D], F16, kind="ExternalInput")
    ind = nc.dram_tensor("ind", [KP, 32], F16, kind="ExternalInput")
    out_t = nc.dram_tensor("out_t", [D, S], F16, kind="ExternalOutput")
    # 4 dumps of the den PSUM bank (2 heads each): [dump, 128, 512]
    den_t = nc.dram_tensor("den_t", [H // 2, 128, 512], F16,
                           kind="ExternalOutput")

    with tile.TileContext(nc) as tc, ExitStack() as ctx:
        sb_k = ctx.enter_context(tc.tile_pool(name="sb_k", bufs=1))
        sb_q = ctx.enter_context(tc.tile_pool(name="sb_q", bufs=1))
        sb_v = ctx.enter_context(tc.tile_pool(name="sb_v", bufs=1))
        sb_ind = ctx.enter_context(tc.tile_pool(name="sb_ind", bufs=1))
        sb_w = ctx.enter_context(tc.tile_pool(name="sb_w", bufs=6))
        sb_out = ctx.enter_context(tc.tile_pool(name="sb_out", bufs=4))
        sb_den = ctx.enter_context(tc.tile_pool(name="sb_den", bufs=2))
        ps_l = ctx.enter_context(
            tc.tile_pool(name="ps_l", bufs=pl_bufs, space="PSUM"))
        ps_o = ctx.enter_context(
            tc.tile_pool(name="ps_o", bufs=po_bufs, space="PSUM"))
        ps_d = ctx.enter_context(tc.tile_pool(name="ps_d", bufs=1, space="PSUM"))

        # ---- bulk input preload (head 0 slices first) ----
        kth_all = sb_k.tile([128, H * KP], F16)   # [d, (h, k)]
        qth_all = sb_q.tile([128, H * S], F16)    # [d, (h, q)]
        vh_all = sb_v.tile([128, kt_tiles * D], F16)  # [k, (kt, d)]
        ind_sb = sb_ind.tile([128, kt_tiles * 32], F16)

        k3 = kth_all[:].rearrange("p (h k) -> p h k", h=H)
        q3 = qth_all[:].rearrange("p (h q) -> p h q", h=H)
        v3 = vh_all[:].rearrange("p (t c) -> p t c", c=D)
        ksrc = k_t.ap().rearrange("(h p) k -> p h k", p=128)
        qsrc = q_t.ap().rearrange("(h p) q -> p h q", p=128)
        nc.sync.dma_start(k3[:, 0:1, :], ksrc[:, 0:1, :])
        nc.sync.dma_start(q3[:, 0:1, :], qsrc[:, 0:1, :])
        nc.sync.dma_start(
            v3[:, 0:1, :],
            v_c.ap()[0:128, :].rearrange("(t p) c -> p t c", p=128),
        )
        nc.sync.dma_start(
            ind_sb[:].rearrange("p (t c) -> p t c", c=32),
            ind.ap().rearrange("(t p) c -> p t c", p=128),
        )
        nc.sync.dma_start(k3[:, 1:, :], ksrc[:, 1:, :])
        nc.sync.dma_start(q3[:, 1:, :], qsrc[:, 1:, :])
        if kt_tiles > 1:
            nc.sync.dma_start(
                v3[:, 1:, :],
                v_c.ap()[128:, :].rearrange("(t p) c -> p t c", p=128),
            )

        s0, s1 = slice(0, 512), slice(512, 1024)
        pd = None

        for h in range(H):
            hs = h * DH
            kth = k3[:, h, :]                     # [128, KP]
            qth = q3[:, h, :]                     # [128, S]

            po = ps_o.tile([128, S], F32, tag="po")    # outT accum [d, q]
            if h % 2 == 0:
                pd = ps_d.tile([128, 512], F32, tag="pd")
            # den rows for this head within pd (2 heads per bank)
            r0 = (h % 2) * 64

            wts = []

            def emit_qk(kt, h=h, kth=kth, qth=qth):
                # one PSUM bank per q-half so 3 pl bufs fit in 3 banks
                pl0 = ps_l.tile([128, 512], F32, tag="pl", name=f"pl_{h}_{kt}a")
                pl1 = ps_l.tile([128, 512], F32, tag="pl", name=f"pl_{h}_{kt}b")
                ks = kt * 128
                kA, kB = slice(ks, ks + 64), slice(ks + 64, ks + 128)
                nc.tensor.matmul(pl0[0:64, :], kth[:, kA], qth[:, s0])
                nc.tensor.matmul(pl1[64:128, :], kth[:, kB], qth[:, s1])
                nc.tensor.matmul(pl0[64:128, :], kth[:, kB], qth[:, s0])
                nc.tensor.matmul(pl1[0:64, :], kth[:, kA], qth[:, s1])
                wt = sb_w.tile([128, S], F16, tag="wt", name=f"wt_{h}_{kt}")
                for pl, sq in ((pl0, s0), (pl1, s1)):
                    if kt in dve_kts:
                        # fast-exp: write fp16 bit pattern via int16 view
                        nc.vector.tensor_scalar(
                            wt[:, sq].bitcast(I16), pl[:], EXP_C1, EXP_C2,
                            mybir.AluOpType.mult, mybir.AluOpType.add,
                        )
                    else:
                        nc.scalar.activation(
                            wt[:, sq], pl[:], mybir.ActivationFunctionType.Exp,
                            scale=SCALE,
                        )
                wts.append(wt)

            emit_qk(0)
            for kt in range(kt_tiles):
                if kt + 1 < kt_tiles:
                    emit_qk(kt + 1)
                wt = wts[kt]
                ks = kt * 128
                dA, dB = slice(ks, ks + 64), slice(ks + 64, ks + 128)
                first, last = kt == 0, kt == kt_tiles - 1
                ic = slice(kt * 32, kt * 32 + 32)
                vA = v3[:, kt, hs:hs + 64]
                vB = v3[:, kt, hs + 64:hs + DH]
                seqs = [
                    (pd[r0:r0 + 32, :], ind_sb[:, ic], wt[:, s0], (0, r0)),
                    (pd[r0 + 32:r0 + 64, :], ind_sb[:, ic], wt[:, s1],
                     (0, r0 + 32)),
                    (po[0:64, s0], vA, wt[:, s0], None),
                    (po[64:128, s1], vB, wt[:, s1], None),
                    (po[64:128, s0], vB, wt[:, s0], None),
                    (po[0:64, s1], vA, wt[:, s1], None),
                ]
                for out_ap, w_ap, r_ap, tp in seqs:
                    nc.tensor.matmul(out_ap, w_ap, r_ap, start=first,
                                     stop=last, tile_position=tp)

            # numerator to SBUF fp16 (DVE), divide on host
            osb = sb_out.tile([128, S], F16, tag="osb")
            nc.vector.tensor_copy(osb[:], po[:])
            nc.sync.dma_start(out_t.ap()[hs:hs + DH, :], osb[:])
            if h % 2 == 1:
                dsb = sb_den.tile([128, 512], F16, tag="dsb")
                nc.scalar.copy(dsb[:], pd[:])
                nc.sync.dma_start(den_t.ap()[h // 2, :, :], dsb[:])

    nc.compile()
    return nc


def kernel(memory, query, seq_mask, b):
    memory = np.ascontiguousarray(memory, dtype=np.float32)
    query = np.ascontiguousarray(query, dtype=np.float32)
    seq_mask = np.asarray(seq_mask)
    assert memory.shape == (B, S, 2 * D) and query.shape == (B, S, D)

    counts = [int(np.count_nonzero(seq_mask[i])) for i in range(B)]
    kp = max(max(counts), 1)
    kp = ((kp + 127) // 128) * 128
    kt_tiles = kp // 128

    key = (kt_tiles, tuple(sorted(OPTS.items())))
    if key not in _NC_CACHE:
        _NC_CACHE[key] = _build(kt_tiles, OPTS)
    nc = _NC_CACHE[key]

    q_t = np.ascontiguousarray(query.transpose(0, 2, 1)).astype(np.float16)
    in_maps = []
    for i in range(B):
        idx = np.flatnonzero(seq_mask[i])
        nb = len(idx)
        ktb = np.zeros((D, kp), dtype=np.float16)
        vcb = np.zeros((kp, D), dtype=np.float16)
        indb = np.zeros((kp, 32), dtype=np.float16)
        if nb:
            ktb[:, :nb] = memory[i, idx, :D].T
            vcb[:nb] = memory[i, idx, D:]
            indb[:nb] = 1.0
        in_maps.append({"q_t": q_t[i], "k_t": ktb, "v_c": vcb, "ind": indb})

    res = run_bass_kernel_spmd(nc, in_maps, list(range(B)))
    out = np.empty((B, S, D), dtype=np.float32)
    for i in range(B):
        num = res.results[i]["out_t"].astype(np.float32)   # [D, S]
        dd = res.results[i]["den_t"].astype(np.float32)    # [4, 128, 512]
        den = np.empty((H, S), dtype=np.float32)
        for hh in range(H):
            blk = dd[hh // 2]
            r0 = (hh % 2) * 64
            den[hh, 0:512] = blk[r0]
            den[hh, 512:1024] = blk[r0 + 32]
        with np.errstate(divide="ignore", invalid="ignore"):
            out[i] = (num.reshape(H, DH, S) / den[:, None, :]).reshape(D, S).T
        if counts[i] == 0:
            # all keys masked: reference softmax degenerates to uniform
            out[i] = memory[i, :, D:].mean(axis=0)[None, :]
    return out


# revision 8
# speedup vs baseline: 1.0024x; 1.0024x over previous
"""Multi-head attention (B=8, H=8, S=1024, d=128) on 8 TRN2 NeuronCores.

Strategy (v2)
-------------
- Data-parallel over batch: core i handles batch i (8 cores, B=8).
- Host-side prep (layout only): per batch, compact keys/values to the
  seq_mask-selected rows (zero-padded to kt_tiles 128-wide k-tiles),
  pre-transpose Q and compacted K so the contraction dim (d) lands on
  SBUF partitions, cast matmul operands to fp16. An indicator matrix
  ind[k, 32] (1 for real keys) rides along for the softmax denominator.
- All inputs are bulk-preloaded into SBUF with a handful of large DMAs
  (head-0 slices first so compute starts early); no per-head DMA stalls.
- Device math per (head h, k-tile kt):
    logitsT[k, q]  = K^T[:, kt].T @ Q^T      (PE, fp16, bf16 PSUM out)
    W^T[k, q]      = exp(logitsT * d^-0.5)   (ACT exp LUT for ~3/5 tiles,
                     DVE fast-exp bit-trick (tensor_scalar -> int16 bits
                     of fp16) for ~2/5 tiles: splits the exp load across
                     two engines; |rel err| <= ~3% on the trick tiles,
                     which averages out across ~500 attended keys)
    outT[d, q]    += V[kt].T   @ W^T         (PE, fp32 PSUM accum)
    den[q]        += ind[kt].T @ W^T         (PE, 2 heads packed per
                                              PSUM bank via partitions)
  Outputs leave as fp16 (numerator and denominator); the division
  happens on the host. The learned scalar bias b cancels in softmax.
- Host-side unshard: out[b] = (numT / den).T (uniform-average fallback
  for a fully-masked batch).
"""
from contextlib import ExitStack

import numpy as np

import concourse.bacc as bacc
import concourse.mybir as mybir
import concourse.tile as tile
from concourse.bass_utils import run_bass_kernel_spmd

F32 = mybir.dt.float32
F16 = mybir.dt.float16
BF16 = mybir.dt.bfloat16
I16 = mybir.dt.int16

B, S, D, H = 8, 1024, 1024, 8
DH = D // H              # 128, head dim = one partition tile
SCALE = float(DH) ** -0.5

# fast-exp bit trick constants (fp16): exp(x) ~= fp16_from_bits(
#   round(x * 1024*log2(e) + (15*1024 - 44)))  -- 44 centers the
# log2(1+m)-m mantissa interpolation error (+-3%, ~zero mean).
EXP_C1 = 1024.0 * 1.4426950408889634 * SCALE
EXP_C2 = 15.0 * 1024.0 - 44.0

_NC_CACHE: dict[tuple, object] = {}

# build options (overridable for profiling experiments)
OPTS: dict = {}


def _build(kt_tiles: int, opts: dict | None = None):
    """Build + compile the per-core kernel for `kt_tiles` 128-wide key tiles."""
    opts = opts or {}
    KP = kt_tiles * 128
    pl_bufs = opts.get("pl_bufs", 2)
    po_bufs = opts.get("po_bufs", 1)

    nc = bacc.Bacc("TRN2", target_bir_lowering=False, debug=False)

    q_t = nc.dram_tensor("q_t", [D, S], F16, kind="ExternalInput")
    k_t = nc.dram_tensor("k_t", [D, KP], F16, kind="ExternalInput")
    v_c = nc.dram_tensor("v_c", [KP, D], F16, kind="ExternalInput")
    ind = nc.dram_tensor("ind", [KP, 32], F16, kind="ExternalInput")
    out_t = nc.dram_tensor("out_t", [D, S], F16, kind="ExternalOutput")
    # 4 dumps of the den PSUM bank (2 heads each): [dump, 128, 512]
    den_t = nc.dram_tensor("den_t", [H // 2, 128, 512], F16,
                           kind="ExternalOutput")

    with tile.TileContext(nc) as tc, ExitStack() as ctx:
        sb_k = ctx.enter_context(tc.tile_pool(name="sb_k", bufs=1))
        sb_q = ctx.enter_context(tc.tile_pool(name="sb_q", bufs=1))
        sb_v = ctx.enter_context(tc.tile_pool(name="sb_v", bufs=1))
        sb_ind = ctx.enter_context(tc.tile_pool(name="sb_ind", bufs=1))
        sb_w = ctx.enter_context(tc.tile_pool(name="sb_w", bufs=6))
        sb_out = ctx.enter_context(tc.tile_pool(name="sb_out", bufs=4))
        sb_den = ctx.enter_context(tc.tile_pool(name="sb_den", bufs=2))
        ps_l = ctx.enter_context(
            tc.tile_pool(name="ps_l", bufs=pl_bufs, space="PSUM"))
        ps_o = ctx.enter_context(
            tc.tile_pool(name="ps_o", bufs=po_bufs, space="PSUM"))
        ps_d = ctx.enter_context(tc.tile_pool(name="ps_d", bufs=1, space="PSUM"))

        # ---- bulk input preload (head 0 slices first) ----
        kth_all = sb_k.tile([128, H * KP], F16)   # [d, (h, k)]
        qth_all = sb_q.tile([128, H * S], F16)    # [d, (h, q)]
        vh_all = sb_v.tile([128, kt_tiles * D], F16)  # [k, (kt, d)]
        ind_sb = sb_ind.tile([128, kt_tiles * 32], F16)

        k3 = kth_all[:].rearrange("p (h k) -> p h k", h=H)
        q3 = qth_all[:].rearrange("p (h q) -> p h q", h=H)
        v3 = vh_all[:].rearrange("p (t c) -> p t c", c=D)
        ksrc = k_t.ap().rearrange("(h p) k -> p h k", p=128)
        qsrc = q_t.ap().rearrange("(h p) q -> p h q", p=128)
        nc.sync.dma_start(k3[:, 0:1, :], ksrc[:, 0:1, :])
        nc.sync.dma_start(q3[:, 0:1, :], qsrc[:, 0:1, :])
        nc.sync.dma_start(
            v3[:, 0:1, :],
            v_c.ap()[0:128, :].rearrange("(t p) c -> p t c", p=128),
        )
        nc.sync.dma_start(
            ind_sb[:].rearrange("p (t c) -> p t c", c=32),
            ind.ap().rearrange("(t p) c -> p t c", p=128),
        )
        nc.sync.dma_start(k3[:, 1:, :], ksrc[:, 1:, :])
        nc.sync.dma_start(q3[:, 1:, :], qsrc[:, 1:, :])
        if kt_tiles > 1:
            nc.sync.dma_start(
                v3[:, 1:, :],
                v_c.ap()[128:, :].rearrange("(t p) c -> p t c", p=128),
            )

        s0, s1 = slice(0, 512), slice(512, 1024)
        pd = None

        for h in range(H):
            hs = h * DH
            kth = k3[:, h, :]                     # [128, KP]
            qth = q3[:, h, :]                     # [128, S]

            po = ps_o.tile([128, S], F32, tag="po")    # outT accum [d, q]
            if h % 2 == 0:
                pd = ps_d.tile([128, 512], F32, tag="pd")
            # den rows for this head within pd (2 heads per bank)
            r0 = (h % 2) * 64

            wts = []

            def emit_qk(kt, h=h, kth=kth, qth=qth):
                pl = ps_l.tile([128, S], F32, tag="pl", name=f"pl_{h}_{kt}")
                ks = kt * 128
                kA, kB = slice(ks, ks + 64), slice(ks + 64, ks + 128)
                nc.tensor.matmul(pl[0:64, s0], kth[:, kA], qth[:, s0])
                nc.tensor.matmul(pl[64:128, s1], kth[:, kB], qth[:, s1])
                nc.tensor.matmul(pl[64:128, s0], kth[:, kB], qth[:, s0])
                nc.tensor.matmul(pl[0:64, s1], kth[:, kA], qth[:, s1])
                wt = sb_w.tile([128, S], F16, tag="wt", name=f"wt_{h}_{kt}")
                nc.scalar.activation(
                    wt[:], pl[:], mybir.ActivationFunctionType.Exp,
                    scale=SCALE,
                )
                wts.append(wt)

            emit_qk(0)
            for kt in range(kt_tiles):
                if kt + 1 < kt_tiles:
                    emit_qk(kt + 1)
                wt = wts[kt]
                ks = kt * 128
                dA, dB = slice(ks, ks + 64), slice(ks + 64, ks + 128)
                first, last = kt == 0, kt == kt_tiles - 1
                ic = slice(kt * 32, kt * 32 + 32)
                vA = v3[:, kt, hs:hs + 64]
                vB = v3[:, kt, hs + 64:hs + DH]
                seqs = [
                    (pd[r0:r0 + 32, :], ind_sb[:, ic], wt[:, s0], (0, r0)),
                    (pd[r0 + 32:r0 + 64, :], ind_sb[:, ic], wt[:, s1],
                     (0, r0 + 32)),
                    (po[0:64, s0], vA, wt[:, s0], None),
                    (po[64:128, s1], vB, wt[:, s1], None),
                    (po[64:128, s0], vB, wt[:, s0], None),
                    (po[0:64, s1], vA, wt[:, s1], None),
                ]
                for out_ap, w_ap, r_ap, tp in seqs:
                    nc.tensor.matmul(out_ap, w_ap, r_ap, start=first,
                                     stop=last, tile_position=tp)

            # numerator to SBUF fp16 (DVE), divide on host
            osb = sb_out.tile([128, S], F16, tag="osb")
            nc.vector.tensor_copy(osb[:], po[:])
            nc.sync.dma_start(out_t.ap()[hs:hs + DH, :], osb[:])
            if h % 2 == 1:
                dsb = sb_den.tile([128, 512], F16, tag="dsb")
                nc.vector.tensor_copy(dsb[:], pd[:])
                nc.sync.dma_start(den_t.ap()[h // 2, :, :], dsb[:])

    nc.compile()
    return nc


def kernel(memory, query, seq_mask, b):
    memory = np.ascontiguousarray(memory, dtype=np.float32)
    query = np.ascontiguousarray(query, dtype=np.float32)
    seq_mask = np.asarray(seq_mask)
    assert memory.shape == (B, S, 2 * D) and query.shape == (B, S, D)

    counts = [int(np.count_nonzero(seq_mask[i])) for i in range(B)]
    kp = max(max(counts), 1)
    kp = ((kp + 127) // 128) * 128
    kt_tiles = kp // 128

    key = (kt_tiles, tuple(sorted(OPTS.items())))
    if key not in _NC_CACHE:
        _NC_CACHE[key] = _build(kt_tiles, OPTS)
    nc = _NC_CACHE[key]

    q_t = np.ascontiguousarray(query.transpose(0, 2, 1)).astype(np.float16)
    in_maps = []
    for i in range(B):
        idx = np.flatnonzero(seq_mask[i])
        nb = len(idx)
        ktb = np.zeros((D, kp), dtype=np.float16)
        vcb = np.zeros((kp, D), dtype=np.float16)
        indb = np.zeros((kp, 32), dtype=np.float16)
        if nb:
            ktb[:, :nb] = memory[i, idx, :D].T
            vcb[:nb] = memory[i, idx, D:]
            indb[:nb] = 1.0
        in_maps.append({"q_t": q_t[i], "k_t": ktb, "v_c": vcb, "ind": indb})

    res = run_bass_kernel_spmd(nc, in_maps, list(range(B)))
    out = np.empty((B, S, D), dtype=np.float32)
    for i in range(B):
        num = res.results[i]["out_t"].astype(np.float32)   # [D, S]
        dd = res.results[i]["den_t"].astype(np.float32)    # [4, 128, 512]
        den = np.empty((H, S), dtype=np.float32)
        for hh in range(H):
            blk = dd[hh // 2]
            r0 = (hh % 2) * 64
            den[hh, 0:512] = blk[r0]
            den[hh, 512:1024] = blk[r0 + 32]
        with np.errstate(divide="ignore", invalid="ignore"):
            out[i] = (num.reshape(H, DH, S) / den[:, None, :]).reshape(D, S).T
        if counts[i] == 0:
            # all keys masked: reference softmax degenerates to uniform
            out[i] = memory[i, :, D:].mean(axis=0)[None, :]
    return out
